# revision 42
# baseline (speedup 1.0000x reference)
"""Trainium2 Bass kernel for nn_Attention_70136815943694.

Attention with the reference's source bug preserved (K uses the V
projection). x:[2,2048,1024], 16 heads x 64 dim. Sharded over 8
NeuronCores as (batch x head-group): core c handles batch c//4 and
heads [4*(c%4) .. 4*(c%4)+3]. Each core's output slice is independent,
so there are no collectives; the host shards inputs and reassembles.

Per-core device pipeline (d-major layouts):
  QT = wqT.T @ xT (+bq)     [256, 2048] bf16   (DVE evac w/ bias)
  KVT = wvT.T @ xT (+bv)    [256, 2048] bf16
  V   = PE-transpose of KVT chunks into [128, 2(head), 128] tiles:
        per head the lhsT is [1 | 63 zeros | V], so the attnT psum
        carries the softmax denominator at partition 0 (read directly
        by the DVE reciprocal) and V output on the 64-aligned
        partitions 64:128 (engine APs over 64 partitions must be
        64-aligned, and two-input DVE ops need equal base partitions).
  per head-pair p, s1 quarter q (512 wide), s2 chunk j (128):
    scores: two K=64 matmuls row-packed via tile_position (0,0)/(64,0)
            sharing one 512-col rhs stream into one [128,1024] psum
    PT: ACT exp ([128,1024], 1.11us) or a one-op DVE Schraudolph exp2
        (fused multiply-add, int16-converting write emitting the bf16
        bit pattern, 1.22us) -- engine chosen per slot (below)
  atH += lhsT.T @ PT_h        [128, 512] psum per head
  group epilogue: boundary slots j=0,1 of the next group run their exp
  on the DVE, freeing the ACT to evacuate the finished psum; the DVE
  reciprocal reads the denom row straight out of psum, gpsimd
  partition-broadcasts it, DVE mul + DMA emit the output. The final
  group instead broadcasts via a K=1 PE matmul into a freed sc bank.

Schedule, two phases:
  p=0 half (slots 0-63): carries ALL projection fill and V transposes
  as per-slot filler chains paced to DMA arrival -- PE-bound at
  ~1.29us/slot. DVE exp only on {0,1,9} boundaries/mid.
  p=1 half (slots 64-127): no fill left; the mi/vt psum pools close at
  slot 64 and their banks become a THIRD sc buffer, deepening the
  scores->exp ring so the sc WAR latency pipelines away; exp alternates
  ACT/DVE (j%2==0 and j!=6 on DVE) and the two engines overlap --
  ~0.84us/slot, vs 1.15us for an ACT-only stream.
Input DMA is striped need-first over both HWDGE rings (measured under
load: sync ~186GB/s, scalar ~97): m0 weights, then x quarters (k0-5
sync / k6-7 scalar), m1 weights last; biases ride the SWDGE. Ident
warm-up matmuls bridge DMA waits so the PE HAM never re-throttles.
attnT consumption is deferred two slots so the exp stream never
queues behind it. HW exec ~175us (from ~179us), rel err 4.3e-3.
"""
import numpy as np
import ml_dtypes

B = 2
S = 2048
D = 1024
NH = 16
HD = 64
N_CORES = 8
HEADS_PER_CORE = 4
DPC = HEADS_PER_CORE * HD  # 256 projection rows per core
P = 128
KC = D // P  # 8 contraction chunks
SC = S // P  # 16 s2 chunks
SQ = 512  # s1 quarter width
NSQ = S // SQ  # 4

_NC_CACHE = {}


def build_nc():
    if "nc" in _NC_CACHE:
        return _NC_CACHE["nc"]
    import concourse.bass as bass
    import concourse.mybir as mybir
    import concourse.tile as tile
    from concourse import bacc
    from concourse.masks import make_identity

    BF16 = mybir.dt.bfloat16
    F16 = mybir.dt.float16
    F32 = mybir.dt.float32
    I16 = mybir.dt.int16
    Act = mybir.ActivationFunctionType
    Alu = mybir.AluOpType
    ts = bass.ts

    # Schraudolph exp on the DVE: exp(0.125*s) = 2^(0.125*log2e*s); the
    # bf16 bits of that are ~ (127 - c + t)*2^7. One fused multiply-add
    # with an int16-converting write emits them directly. c=0.0579
    # centers the sawtooth error (~1.8% rms) -- softmax tolerant.
    SCH_K = 0.125 * 1.4426950408889634 * (1 << 7)
    SCH_B = (127.0 - 0.0579) * (1 << 7) + 0.5

    def is_dve(p, q, j):
        # p=1 groups carry no projection fill, so the DVE is nearly
        # idle: alternating the exp between ACT and DVE every other
        # slot lets consecutive exps overlap (sc is double-buffered)
        # and the cadence approaches the PE floor instead of the
        # 1.15us ACT op time. p=0 groups keep the DVE for projection
        # evacuations; boundary slots j=0,1 free the ACT for the
        # previous group's psum evacuation.
        if p == 1:
            return j % 2 == 0 and j != 6
        if (p, q) == (0, 0):
            return j in (5, 9, 13)
        return j in (0, 1, 9)

    nc = bacc.Bacc(None, target_bir_lowering=False, debug=False)
    xT_d = nc.declare_dram_parameter("xT", [P, 4, KC, 512], BF16, isOutput=False)
    wqT_d = nc.declare_dram_parameter("wqT", [P, 2, KC, P], BF16, isOutput=False)
    wvT_d = nc.declare_dram_parameter("wvT", [P, 2, KC, P], BF16, isOutput=False)
    bq_d = nc.declare_dram_parameter("bq", [DPC, 1], F32, isOutput=False)
    bv_d = nc.declare_dram_parameter("bv", [DPC, 1], F32, isOutput=False)
    out_d = nc.declare_dram_parameter("out", [DPC, S], F32, isOutput=True)

    with tile.TileContext(nc) as tc:
        with (
            tc.tile_pool(name="persist", bufs=1) as persist,
            tc.tile_pool(name="pt", bufs=8) as pt_pool,
            tc.tile_pool(name="epi", bufs=4) as epi_pool,
            tc.tile_pool(name="vstage", bufs=3) as vstage_pool,
        ):
            # warm the ACT exp table set at t~0 so the one-time table load
            # overlaps the input DMAs
            warm = persist.tile([1, 8], F32, tag="warm")
            nc.vector.memset(warm[:], 0.0)
            nc.scalar.activation(warm[:], warm[:], Act.Exp, scale=1.0)

            ident = persist.tile([P, P], BF16, tag="ident")
            make_identity(nc, ident[:])
            ones16 = persist.tile([1, HD], F16, tag="ones16")
            nc.vector.memset(ones16[:], 1.0)

            xt_big = persist.tile([P, 4, KC, 512], BF16, name="xt", tag="xt")

            def xt_q(k, nq):  # [128, 512] slice: k-chunk, column quarter nq
                return xt_big[:, nq, k, :]

            wq_big = persist.tile([P, 2, KC, P], BF16, name="wq", tag="wq")
            wv_big = persist.tile([P, 2, KC, P], BF16, name="wv", tag="wv")
            wq_sb = [[wq_big[:, m, k, :] for m in range(2)] for k in range(KC)]
            wv_sb = [[wv_big[:, m, k, :] for m in range(2)] for k in range(KC)]
            bq_sb = [
                persist.tile([P, 1], F32, name=f"bq{m}", tag=f"bq{m}")
                for m in range(2)
            ]
            bv_sb = [
                persist.tile([P, 1], F32, name=f"bv{m}", tag=f"bv{m}")
                for m in range(2)
            ]
            # ---- input loads: need-first across both HWDGE rings
            # (measured under compute contention: sync ~186GB/s, scalar
            # ~97): wq m0 + k0-5 of each quarter on sync, wv m0 + k6-7
            # on scalar, so each x quarter completes in ~4us; biases on
            # the SWDGE; m1 weights last.
            nc.sync.dma_start(wq_big[:, 0], wqT_d[:, 0])
            nc.scalar.dma_start(wv_big[:, 0], wvT_d[:, 0])
            nc.gpsimd.dma_start(bq_sb[0][:], bq_d[0:P, :])
            nc.gpsimd.dma_start(bv_sb[0][:], bv_d[0:P, :])
            nc.gpsimd.dma_start(bq_sb[1][:], bq_d[P:DPC, :])
            nc.gpsimd.dma_start(bv_sb[1][:], bv_d[P:DPC, :])
            # quarter 0 in finer pieces so the first projection chases
            # chunk arrival
            nc.sync.dma_start(xt_big[:, 0, 0:3, :], xT_d[:, 0, 0:3, :])
            nc.sync.dma_start(xt_big[:, 0, 3:6, :], xT_d[:, 0, 3:6, :])
            nc.scalar.dma_start(xt_big[:, 0, 6:8, :], xT_d[:, 0, 6:8, :])
            for nq in range(1, 4):
                nc.sync.dma_start(xt_big[:, nq, 0:6, :], xT_d[:, nq, 0:6, :])
                nc.scalar.dma_start(xt_big[:, nq, 6:8, :], xT_d[:, nq, 6:8, :])
            nc.sync.dma_start(wq_big[:, 1], wqT_d[:, 1])
            nc.scalar.dma_start(wv_big[:, 1], wvT_d[:, 1])

            qT_sb = [
                persist.tile([P, S], BF16, name=f"qT{m}", tag=f"qT{m}")
                for m in range(2)
            ]
            kvT_sb = [
                persist.tile([P, S], BF16, name=f"kvT{m}", tag=f"kvT{m}")
                for m in range(2)
            ]
            # v3[p][j]: [128, 2, 128] per head hl: col 0 = 1 (denom row
            # source), cols 1:64 = 0 (padding so V lands on the 64-
            # aligned output partitions 64:128 the epilogue mul reads),
            # cols 64:128 = V
            v3 = [
                [
                    persist.tile(
                        [P, 2, P], BF16,
                        name=f"v{p}_{j}", tag=f"v{p}_{j}",
                    )
                    for j in range(SC)
                ]
                for p in range(2)
            ]
            for p in range(2):
                for j in range(SC):
                    nc.vector.memset(v3[p][j][:, :, 0:64], 0.0)
                    nc.vector.memset(v3[p][j][:, :, 0:1], 1.0)

            def proj512(w_sb, dst, bias, m, c0, psum_pool, mpy):
                """One 512-col slice [c0:c0+512] of a projection m-chunk,
                yielding after every mpy contraction chunks; the bias-add
                evacuation rides the final yield."""
                ps = psum_pool.tile([P, 512], F32, tag="mi", name="pp")
                nq = c0 // 512
                for k in range(KC):
                    nc.tensor.matmul(
                        ps[:],
                        w_sb[k][m],
                        xt_q(k, nq),
                        start=(k == 0),
                        stop=(k == KC - 1),
                    )
                    if k % mpy == mpy - 1 and k != KC - 1:
                        yield
                nc.vector.tensor_scalar_add(dst[:, ts(nq, 512)], ps[:], bias[:])
                yield

            def vtrans_dma(p, j0, j1, per=2):
                """V transpose on the DMA xbar instead of the PE: the
                transposed [128,128] pair lands in an SBUF staging tile,
                and a 4x-mode bf16 copy repacks it behind the ones/zero
                columns of v3."""
                for j in range(j0, j1):
                    vs = vstage_pool.tile([P, 2, HD], BF16, tag="vs", name="vs")
                    nc.scalar.dma_start_transpose(
                        vs[:, :, :], kvT_sb[p][:, ts(j, P)]
                    )
                    nc.vector.tensor_copy(v3[p][j][:, :, 64:128], vs[:, :, :])
                    if (j - j0) % per == per - 1:
                        yield
                yield

            def vtrans(p, j0, j1, psum_pool, per=2):
                """PE-transpose KVT chunks into v3 tiles, `per` chunks per
                step. The evacuation is a single two-level-AP copy placing
                both heads' columns after the ones column."""
                for j in range(j0, j1):
                    pst = psum_pool.tile(
                        [P, 2, HD], BF16, tag="vt", name="vt",
                        padded_shape=[P, 2, 512],
                    )
                    nc.tensor.transpose(
                        pst[:, :, :], kvT_sb[p][:, ts(j, P)], ident[:]
                    )
                    nc.vector.tensor_copy(v3[p][j][:, :, 64:128], pst[:, :, :])
                    if (j - j0) % per == per - 1:
                        yield
                yield

            # ---- prologue: PE warm-up while the first DMAs land, then
            # the m0 q0 Q/KV projections interleaved per k so only two
            # matmuls + evacs remain after the last x chunk arrives, then
            # the first two V transposes (attnT j0/j1 consumers).
            with tc.tile_pool(name="psum_pro", bufs=2, space="PSUM") as psum_pro:
                wps = psum_pro.tile([P, 512], F32, tag="warm", name="wps")
                for i in range(30):
                    nc.tensor.matmul(
                        wps[:, 0:P], ident[:], ident[:], start=True, stop=True
                    )
                ps_q = psum_pro.tile([P, 512], F32, tag="mi", name="ppq")
                ps_v = psum_pro.tile([P, 512], F32, tag="mi", name="ppv")
                for k in range(KC):
                    if k == 3:
                        # bridge the wait for xq0's second DMA piece with
                        # ident matmuls so the PE HAM stays at full clock
                        for _ in range(20):
                            nc.tensor.matmul(
                                wps[:, 0:P], ident[:], ident[:],
                                start=True, stop=True,
                            )
                    nc.tensor.matmul(
                        ps_q[:], wq_sb[k][0], xt_q(k, 0),
                        start=(k == 0), stop=(k == KC - 1),
                    )
                    nc.tensor.matmul(
                        ps_v[:], wv_sb[k][0], xt_q(k, 0),
                        start=(k == 0), stop=(k == KC - 1),
                    )
                nc.vector.tensor_scalar_add(qT_sb[0][:, 0:512], ps_q[:], bq_sb[0][:])
                nc.vector.tensor_scalar_add(kvT_sb[0][:, 0:512], ps_v[:], bv_sb[0][:])
                nc.vector.tensor_copy(warm[:], wps[0:1, 0:8])
                for _ in vtrans(0, 0, 2, psum_pro):
                    pass

            # ---- attention ---------------------------------------------------
            from contextlib import ExitStack
            p0_pools = ExitStack()
            with (
                tc.tile_pool(name="psum_sc", bufs=2, space="PSUM") as psum_sc,
                tc.tile_pool(name="psum_at", bufs=2, space="PSUM") as psum_at,
            ):
                psum_mi = p0_pools.enter_context(
                    tc.tile_pool(name="psum_mi", bufs=1, space="PSUM")
                )
                psum_vt = p0_pools.enter_context(
                    tc.tile_pool(name="psum_vt", bufs=1, space="PSUM")
                )

                def seq(*parts):
                    for g in parts:
                        for _ in g:
                            yield

                def skip(n):
                    # hold a chain's emission back so matmuls don't enter
                    # the PE FIFO ahead of their x-quarter's DMA arrival
                    for _ in range(n):
                        yield

                # filler pieces per group: (start_slot_in_group, gen),
                # placed so each chain step's data (x quarter via DMA, or
                # a projection's evacuation) lands just before emission,
                # and spread ~1 step/slot after group (0,0).
                def fillers_for(p, q):
                    if (p, q) == (0, 0):
                        return [
                            (0, vtrans(0, 2, 4, psum_vt)),
                            (2, seq(
                                proj512(wv_sb, kvT_sb[0], bv_sb[0], 0, 512, psum_mi, 4),
                                skip(2),
                                proj512(wv_sb, kvT_sb[0], bv_sb[0], 0, 1024, psum_mi, 4),
                                skip(2),
                                proj512(wv_sb, kvT_sb[0], bv_sb[0], 0, 1536, psum_mi, 4),
                                skip(1),
                                proj512(wq_sb, qT_sb[0], bq_sb[0], 0, 512, psum_mi, 4),
                            )),
                            (4, vtrans(0, 4, 8, psum_vt)),
                            (8, vtrans(0, 8, 12, psum_vt)),
                            (12, vtrans(0, 12, 16, psum_vt)),
                        ]
                    if (p, q) == (0, 1):
                        return [
                            (0, seq(
                                proj512(wq_sb, qT_sb[0], bq_sb[0], 0, 1024, psum_mi, 2),
                                proj512(wq_sb, qT_sb[0], bq_sb[0], 0, 1536, psum_mi, 2),
                                proj512(wv_sb, kvT_sb[1], bv_sb[1], 1, 0, psum_mi, 2),
                            )),
                            (12, vtrans(1, 0, 4, psum_vt)),
                        ]
                    if (p, q) == (0, 2):
                        return [
                            (0, seq(
                                proj512(wv_sb, kvT_sb[1], bv_sb[1], 1, 512, psum_mi, 2),
                                proj512(wv_sb, kvT_sb[1], bv_sb[1], 1, 1024, psum_mi, 2),
                            )),
                            (4, vtrans(1, 4, 8, psum_vt)),
                            (8, vtrans(1, 8, 12, psum_vt)),
                            (8, proj512(wq_sb, qT_sb[1], bq_sb[1], 1, 1024, psum_mi, 2)),
                        ]
                    if (p, q) == (0, 3):
                        # all remaining fill must exhaust by slot 63: the
                        # mi/vt pools close there to lend their banks to
                        # the p=1 half's third sc buffer
                        return [
                            (0, seq(
                                proj512(wv_sb, kvT_sb[1], bv_sb[1], 1, 1536, psum_mi, 2),
                                proj512(wq_sb, qT_sb[1], bq_sb[1], 1, 0, psum_mi, 2),
                                proj512(wq_sb, qT_sb[1], bq_sb[1], 1, 512, psum_mi, 2),
                                proj512(wq_sb, qT_sb[1], bq_sb[1], 1, 1536, psum_mi, 2),
                            )),
                            (4, vtrans(1, 12, 16, psum_vt, per=4)),
                        ]
                    return []

                def run_attn(item):
                    _, pp, pq, pj, pat, ppt, hls = item
                    for hl in hls:
                        nc.tensor.matmul(
                            pat[hl][:],
                            v3[pp][pj][:, hl, 0:P],
                            ppt[:, ts(hl, SQ)],
                            start=(pj == 0),
                            stop=(pj == SC - 1),
                        )

                def emit_asb(at):
                    # ACT evacuates the finished group's psum in the
                    # boundary DVE-exp windows where it is otherwise idle
                    asb = []
                    for hl in range(2):
                        t = epi_pool.tile([HD, SQ], F32, tag="asb", name="asb")
                        nc.scalar.copy(t[:], at[hl][64:128, :])
                        asb.append(t)
                    return asb

                def emit_out(p, q, asb, rcs, hl):
                    ot = epi_pool.tile([HD, SQ], F32, tag="ot", name="ot")
                    nc.vector.tensor_mul(ot[:], asb[hl][:], rcs[hl][:])
                    nc.sync.dma_start(out_d[ts(2 * p + hl, HD), ts(q, SQ)], ot[:])

                def tail_epilogue(p, q, at):
                    # ACT and PE idle after the final exp: reciprocal reads
                    # the denom row straight from psum, then a K=1 PE
                    # broadcast matmul replaces the gpsimd broadcast.
                    rc16 = []
                    for hl in range(2):
                        rc32 = epi_pool.tile([1, SQ], F32, tag="rc1", name="r32")
                        nc.vector.reciprocal_approx_fast(rc32[:], at[hl][0:1, :])
                        t16 = epi_pool.tile([1, SQ], F16, tag="r16", name="r16")
                        nc.vector.tensor_copy(t16[:], rc32[:])
                        rc16.append(t16)
                    asb = emit_asb(at)
                    for hl in range(2):
                        bcp = psum_mi.tile([P, SQ], F32, tag="mi", name="bcp")
                        nc.tensor.matmul(
                            bcp[0:HD, :], ones16[:], rc16[hl][:],
                            start=True, stop=True,
                        )
                        ot = epi_pool.tile([HD, SQ], F32, tag="ot", name="ot")
                        nc.vector.tensor_mul(ot[:], asb[hl][:], bcp[0:HD, :])
                        nc.sync.dma_start(
                            out_d[ts(2 * p + hl, HD), ts(q, SQ)], ot[:]
                        )

                slots = [(p, q, j) for p in range(2) for q in range(NSQ)
                         for j in range(SC)]
                actives = []  # (abs_start_idx, gen)
                at = None
                prev_at = None
                prev_pq = None
                pending = []   # (due_idx, p, q, j, at, pt, hls)
                epi_tasks = []  # (due_idx, fn)

                for idx, (p, q, j) in enumerate(slots):
                    if j == 0:
                        actives += [(idx + s, g) for s, g in fillers_for(p, q)]
                        prev_at, at = at, [
                            psum_at.tile([P, SQ], F32, tag="at", name="at")
                            for _ in range(2)
                        ]
                    if idx == 64:
                        # release the fill pools; a third sc buffer in
                        # their banks deepens the scores->exp ring so the
                        # sc WAR latency pipelines away in the p=1 half
                        p0_pools.close()
                        sc3_pool = p0_pools.enter_context(
                            tc.tile_pool(name="psum_sc3", bufs=1, space="PSUM")
                        )
                        sc_ring = [
                            psum_sc.tile([P, 1024], F32, tag="sc", name="sc"),
                            psum_sc.tile([P, 1024], F32, tag="sc", name="sc"),
                            sc3_pool.tile([P, 1024], F32, tag="sc3", name="sc3"),
                        ]
                    dve = is_dve(p, q, j)
                    pt = pt_pool.tile([P, 1024], BF16, tag="pt", name="pt")
                    if idx < 64:
                        sc = psum_sc.tile([P, 1024], F32, tag="sc", name="sc")
                    else:
                        sc = sc_ring[idx % 3]
                    # the first slots' scores get scheduler priority 0 so
                    # the static schedule doesn't order the slot-2+ filler
                    # projections ahead of them (that ordering, not data
                    # readiness, is what delays the first exp)
                    from contextlib import nullcontext
                    prio = tc.high_priority() if idx < 4 else nullcontext()
                    with prio:
                        for hl in range(2):
                            nc.tensor.matmul(
                                sc[:, ts(hl, SQ)],
                                kvT_sb[p][hl * HD : (hl + 1) * HD, ts(j, P)],
                                qT_sb[p][hl * HD : (hl + 1) * HD, ts(q, SQ)],
                                start=True,
                                stop=True,
                                tile_position=(hl * HD, 0),
                            )
                    due = [x for x in pending if x[0] <= idx]
                    pending = [x for x in pending if x[0] > idx]
                    for item in due:
                        run_attn(item)
                    if j == 0 and prev_at is not None:
                        pp_, pq_ = prev_pq
                        def mk(pp2, pq2, at2):
                            holder = {}
                            def do_evac():
                                # runs at idx+1, after the deferred attnT
                                # j15 (stop) has been emitted
                                asb2 = emit_asb(at2)
                                rcs = []
                                for hl in range(2):
                                    rc1 = epi_pool.tile(
                                        [1, SQ], F32, tag="rc1", name="rc1"
                                    )
                                    nc.vector.reciprocal_approx_fast(
                                        rc1[:], at2[hl][0:1, :]
                                    )
                                    rcb = epi_pool.tile(
                                        [HD, SQ], F32, tag="rcb", name="rcb"
                                    )
                                    nc.gpsimd.partition_broadcast(rcb[:], rc1[:])
                                    rcs.append(rcb)
                                holder["asb"] = asb2
                                holder["rcs"] = rcs
                            def do_out0():
                                emit_out(pp2, pq2, holder["asb"], holder["rcs"], 0)
                            def do_out1():
                                emit_out(pp2, pq2, holder["asb"], holder["rcs"], 1)
                            return do_evac, do_out0, do_out1
                        f_evac, f_o0, f_o1 = mk(pp_, pq_, prev_at)
                        epi_tasks.append((idx + 1, f_evac))
                        epi_tasks.append((idx + 4, f_o0))
                        epi_tasks.append((idx + 5, f_o1))
                    with tc.high_priority() if idx < 4 else nullcontext():
                        if dve:
                            nc.vector.tensor_scalar(
                                pt[:].bitcast(I16),
                                sc[:], SCH_K, SCH_B, Alu.mult, Alu.add,
                            )
                        else:
                            nc.scalar.activation(
                                pt[:], sc[:], Act.Exp, scale=0.125
                            )
                    # filler: advance every startable chain one step
                    still = []
                    for start_i, g in actives:
                        if start_i <= idx:
                            try:
                                next(g)
                                still.append((start_i, g))
                            except StopIteration:
                                pass
                        else:
                            still.append((start_i, g))
                    actives = still
                    edue = [x for x in epi_tasks if x[0] <= idx]
                    epi_tasks = [x for x in epi_tasks if x[0] > idx]
                    for _, fn in edue:
                        fn()
                    pending.append((idx + 2, p, q, j, at, pt, (0, 1)))
                    if j == 0:
                        prev_pq = (p, q)
                for item in sorted(pending, key=lambda x: x[0]):
                    run_attn(item)
                for _, fn in sorted(epi_tasks, key=lambda x: x[0]):
                    fn()
                rc16_t = []
                for hl in range(2):
                    rc1 = epi_pool.tile([1, SQ], F32, tag="rc1", name="rc1")
                    nc.vector.reciprocal_approx_fast(rc1[:], at[hl][0:1, :])
                    t16 = epi_pool.tile([1, SQ], F16, tag="r16", name="r16")
                    nc.vector.tensor_copy(t16[:], rc1[:])
                    rc16_t.append(t16)
                asb_t = emit_asb(at)
                for hl in range(2):
                    nc.tensor.matmul(
                        sc_ring[hl][0:HD, 0:SQ], ones16[:], rc16_t[hl][:],
                        start=True, stop=True,
                    )
                    ot = epi_pool.tile([HD, SQ], F32, tag="ot", name="ot")
                    nc.vector.tensor_mul(
                        ot[:], asb_t[hl][:], sc_ring[hl][0:HD, 0:SQ]
                    )
                    nc.sync.dma_start(out_d[ts(2 + hl, HD), ts(3, SQ)], ot[:])
                p0_pools.close()

    nc.compile()
    _NC_CACHE["nc"] = nc
    return nc


def shard_inputs(x, Wq, bq, Wv, bv):
    bf16 = ml_dtypes.bfloat16
    x = np.asarray(x, dtype=np.float32)
    Wq = np.asarray(Wq, dtype=np.float32)
    bq = np.asarray(bq, dtype=np.float32)
    Wv = np.asarray(Wv, dtype=np.float32)
    bv = np.asarray(bv, dtype=np.float32)
    in_maps = []
    # xT host layout [P, 4, KC, 512]: xT[p, nq, k, s] = x[b][nq*512+s, k*128+p]
    xT = [
        np.ascontiguousarray(
            x[b].T.reshape(KC, P, 4, 512).transpose(1, 2, 0, 3)
        ).astype(bf16)
        for b in range(B)
    ]
    for c in range(N_CORES):
        b, g = divmod(c, N_CORES // B)
        heads = [HEADS_PER_CORE * g + hl for hl in range(HEADS_PER_CORE)]
        perm = np.array([i * NH + h for h in heads for i in range(HD)])
        # wT host layout [P, 2, KC, 128]: wT[p, m, k, j] = W[perm[m*128+j], k*128+p]
        in_maps.append(
            {
                "xT": xT[b],
                "wqT": np.ascontiguousarray(
                    Wq[perm, :].T.reshape(KC, P, 2, P).transpose(1, 2, 0, 3)
                ).astype(bf16),
                "wvT": np.ascontiguousarray(
                    Wv[perm, :].T.reshape(KC, P, 2, P).transpose(1, 2, 0, 3)
                ).astype(bf16),
                "bq": np.ascontiguousarray(bq[perm].reshape(DPC, 1)),
                "bv": np.ascontiguousarray(bv[perm].reshape(DPC, 1)),
            }
        )
    return in_maps


def assemble(results):
    out = np.empty((B, S, D), dtype=np.float32)
    for c in range(N_CORES):
        b, g = divmod(c, N_CORES // B)
        out[b][:, g * DPC : (g + 1) * DPC] = results[c]["out"].T
    return out


def kernel(x, Wq, bq, Wv, bv):
    from concourse.bass_utils import run_bass_kernel_spmd

    nc = build_nc()
    in_maps = shard_inputs(x, Wq, bq, Wv, bv)
    res = run_bass_kernel_spmd(nc, in_maps, core_ids=list(range(N_CORES)))
    return assemble(res.results)


if __name__ == "__main__":
    rng = np.random.default_rng(0)
    inputs = {
        "x": rng.standard_normal((B, S, D), dtype=np.float32),
        "Wq": (rng.standard_normal((D, D), dtype=np.float32) / 32.0),
        "bq": rng.standard_normal(D, dtype=np.float32) * 0.02,
        "Wv": (rng.standard_normal((D, D), dtype=np.float32) / 32.0),
        "bv": rng.standard_normal(D, dtype=np.float32) * 0.02,
    }
    out = kernel(**inputs)
    print("kernel ran, out shape:", out.shape)
